# revision 1
# baseline (speedup 1.0000x reference)
"""Trainium2 Bass kernel for nn_AttnNet: attention-pooling over sequence.

Reference computation (per batch b):
    act    = tanh(X @ W.T + b)          # [S, H]
    scores = act @ context              # [S]
    w      = exp(scores * mask)         # masked_fill(-1e-32) == *mask (exp(0)=1)
    out    = (X.T @ w) / sum(w)         # [H]

Sharding: pure data-parallel, 4 batches per core across 8 cores.

Device layout (per core), all X data in bf16:
    xt   [BPC, KC, 128, S]  bf16  xt[b,k,p,s] = X[b, s, 128k+p]  (X^T, h on partitions)
    xn   [BPC, NXT, 128, 16, H] bf16  (X natural, s on partitions, grouped per 2048-seq half)
    wt   [KC, 128, H]       bf16  wt[k,p,o]   = W[o, 128k+p]     (W^T)
    bias [128, MC]          f32   bias[p,m]   = b[128m+p]
    ctx  [128, MC]          bf16  ctx[p,m]    = context[128m+p]
    mask [BPC, S]           f32
outputs:
    num  [BPC, 4, 512] f32  4 col-group partial pooled rows (host: sum axis=1, divide)
    den  [BPC, NSG]    f32  per-512-chunk partial softmax denominators (host: sum)

Pipeline per (batch, half=2048 seq; subgroups g0..g3 of 512):
    PE : act^T[o,s] psum = sum_k wt[k,m]^T @ xt[k]     (bf16, 16 MM per subgroup)
    ACT: act = tanh(psum + bias[m])                    (per-partition bias fusion)
    PE : scores col-tiled: 4 subgroups concurrently via tile_position=(0,32j)
    DVE: masked = scores * mask          ACT: w = exp(masked), accum_out -> den
    DMA: w row -> DRAM scratch -> read back as columns [128, 16]
    PE : pooling col-tiled: pool_ps[32cc] += w_col^T @ xn[chunk]  (M=1 MMs, x4 concurrent,
         emitted one half late so the w DMA bounce latency hides behind act matmuls)
"""

import numpy as np
import ml_dtypes

import concourse.bass as bass
import concourse.tile as tile
from concourse import bacc, mybir
from concourse.bass_utils import run_bass_kernel_spmd

N_CORES = 8
B, S, H = 32, 4096, 512
BPC = B // N_CORES
P = 128
KC = H // P
MC = H // P
SG = 512
NSG = S // SG
NCH = S // P         # 32 s-chunks per batch (pooling granularity)
XT_TILE = 2048       # seq extent of one SBUF tile ("half")
NXT = S // XT_TILE
GPH = XT_TILE // SG  # subgroups per half = 4

F32 = mybir.dt.float32
BF16 = mybir.dt.bfloat16
BF = ml_dtypes.bfloat16

TRACE = False
LAST = {}


def build():
    nc = bacc.Bacc("TRN2", target_bir_lowering=False, num_devices=N_CORES)
    xt_d = nc.declare_dram_parameter("xt", [BPC, KC, P, S], BF16, isOutput=False)
    xn_d = nc.declare_dram_parameter("xn", [BPC, NXT, P, 4 * GPH, H], BF16, isOutput=False)
    wt_d = nc.declare_dram_parameter("wt", [KC, P, H], BF16, isOutput=False)
    bias_d = nc.declare_dram_parameter("bias", [P, MC], F32, isOutput=False)
    ctx_d = nc.declare_dram_parameter("ctx", [P, MC], BF16, isOutput=False)
    mask_d = nc.declare_dram_parameter("mask", [BPC, S], BF16, isOutput=False)
    num_d = nc.declare_dram_parameter("num", [BPC, 4, SG], F32, isOutput=True)
    den_d = nc.declare_dram_parameter("den", [BPC, NSG], F32, isOutput=True)

    Tanh = mybir.ActivationFunctionType.Tanh
    Exp = mybir.ActivationFunctionType.Exp

    with tile.TileContext(nc) as tc:
        with (
            tc.tile_pool(name="singles", bufs=1) as singles,
            tc.tile_pool(name="xtp", bufs=3) as xtp,
            tc.tile_pool(name="xnp", bufs=3) as xnp,
            tc.tile_pool(name="actpool", bufs=6) as actpool,
            tc.tile_pool(name="maskpool", bufs=2) as maskpool,
            tc.tile_pool(name="mskp", bufs=4) as mskp,
            tc.tile_pool(name="rows", bufs=6) as rows,
            tc.tile_pool(name="wcols", bufs=8) as wcols,
            tc.tile_pool(name="numr", bufs=6) as numr,
            tc.tile_pool(name="dens", bufs=2) as dens,
            tc.tile_pool(name="scratchd", bufs=6, space="DRAM") as scratchd,
            tc.tile_pool(name="actps", bufs=4, space="PSUM") as actps,
            tc.tile_pool(name="scps", bufs=2, space="PSUM") as scps,
            tc.tile_pool(name="poolps", bufs=2, space="PSUM") as poolps,
        ):
            wt_sb = singles.tile([P, KC, H], BF16)
            for k in range(KC):
                nc.sync.dma_start(out=wt_sb[:, k, :], in_=wt_d.ap()[k])
            ctx_sb = singles.tile([P, MC], BF16)
            nc.sync.dma_start(out=ctx_sb[:, :], in_=ctx_d.ap())
            bias_sb = singles.tile([P, MC], F32)
            nc.sync.dma_start(out=bias_sb[:, :], in_=bias_d.ap())

            pending_pool = None
            for b in range(BPC):
                mask_sb = maskpool.tile([1, S], BF16, tag="mask")
                nc.sync.dma_start(out=mask_sb[:, :], in_=mask_d.ap()[b : b + 1, :])
                den_sb = dens.tile([1, NSG], F32, tag="den")
                nc.vector.memset(den_sb[:, :], 0.0)
                pool_ps = poolps.tile([P, SG], F32, tag="pool")

                for half in range(NXT):
                    xt_sb = xtp.tile([P, KC, XT_TILE], BF16, tag="xt")
                    if b == 0 and half == 0:
                        for gl0 in range(GPH):
                            for k in range(KC):
                                nc.sync.dma_start(
                                    out=xt_sb[:, k, gl0 * SG : (gl0 + 1) * SG],
                                    in_=xt_d.ap()[b, k, :, gl0 * SG : (gl0 + 1) * SG],
                                )
                    else:
                        for k in range(KC):
                            nc.sync.dma_start(
                                out=xt_sb[:, k, :],
                                in_=xt_d.ap()[b, k, :, half * XT_TILE : (half + 1) * XT_TILE],
                            )
                    xn_sb = xnp.tile([P, 4 * GPH, SG], BF16, tag="xn")
                    nc.sync.dma_start(out=xn_sb[:, :, :], in_=xn_d.ap()[b, half])

                    act_tiles = []
                    for gl in range(GPH):
                        ssl = slice(gl * SG, (gl + 1) * SG)
                        act_sb = actpool.tile([P, MC, SG], BF16, tag="act")
                        act_tiles.append(act_sb)
                        for m in range(MC):
                            ps = actps.tile([P, SG], F32, tag="ps")
                            for k in range(KC):
                                nc.tensor.matmul(
                                    ps[:, :],
                                    lhsT=wt_sb[:, k, m * P : (m + 1) * P],
                                    rhs=xt_sb[:, k, ssl],
                                    start=(k == 0),
                                    stop=(k == KC - 1),
                                )
                            nc.scalar.activation(
                                out=act_sb[:, m, :],
                                in_=ps[:, :],
                                func=Tanh,
                                bias=bias_sb[:, m : m + 1],
                            )

                    # scores for the 4 subgroups of this half, col-tiled
                    sps = scps.tile([P, SG], F32, tag="sps")
                    for m in range(MC):
                        for j in range(GPH):
                            nc.tensor.matmul(
                                sps[32 * j : 32 * j + 1, :],
                                lhsT=ctx_sb[:, m : m + 1],
                                rhs=act_tiles[j][:, m, :],
                                start=(m == 0),
                                stop=(m == MC - 1),
                                tile_position=(0, 32 * j),
                            )

                    if pending_pool is not None:
                        pending_pool()
                        pending_pool = None

                    final_half = b == BPC - 1 and half == NXT - 1
                    w_cols = wcols.tile([P, 4 * GPH], BF16, tag="wc")

                    def pool_wave(gl2, pps=pool_ps, wcs=w_cols, xn=xn_sb, hh=half):
                        for cc in range(4):
                            ci = gl2 * 4 + cc
                            nc.tensor.matmul(
                                pps[32 * cc : 32 * cc + 1, :],
                                lhsT=wcs[:, ci : ci + 1],
                                rhs=xn[:, ci, :],
                                start=(hh == 0 and gl2 == 0),
                                stop=(hh == NXT - 1 and gl2 == GPH - 1),
                                tile_position=(0, 32 * cc),
                                skip_group_check=True,
                            )

                    def emit_num(pps=pool_ps, bb=b):
                        for j in range(4):
                            nr = numr.tile([1, SG], F32, tag="nr")
                            nc.vector.tensor_copy(nr[:, :], pps[32 * j : 32 * j + 1, :])
                            nc.sync.dma_start(
                                out=num_d.ap()[bb, j : j + 1, :], in_=nr[:, :]
                            )

                    for gl in range(GPH):
                        g = half * GPH + gl
                        msk = mskp.tile([1, SG], F32, tag="msk")
                        nc.vector.tensor_mul(
                            msk[:, :],
                            sps[32 * gl : 32 * gl + 1, :],
                            mask_sb[:, g * SG : (g + 1) * SG],
                        )
                        w_row = rows.tile([1, SG], BF16, tag="w")
                        nc.scalar.activation(
                            out=w_row[:, :],
                            in_=msk[:, :],
                            func=Exp,
                            accum_out=den_sb[:, g : g + 1],
                        )
                        wsc = scratchd.tile([1, SG], BF16, tag="wsc")
                        nc.sync.dma_start(out=wsc[:, :], in_=w_row[:, :])
                        nc.sync.dma_start(
                            out=w_cols[:, gl * 4 : (gl + 1) * 4],
                            in_=wsc[:, :].rearrange("a (c p) -> (a p) c", p=P),
                        )
                        if final_half:
                            pool_wave(gl)

                    if final_half:
                        emit_num()
                    else:

                        def emit_pool(waves=pool_wave, num_fn=emit_num, hh=half):
                            for gl2 in range(GPH):
                                waves(gl2)
                            if hh == NXT - 1:
                                num_fn()

                        pending_pool = emit_pool

                nc.sync.dma_start(out=den_d.ap()[b : b + 1, :], in_=den_sb[:, :])

    nc.compile()
    return nc


_NC_CACHE = {}


def _get_nc():
    if "nc" not in _NC_CACHE:
        _NC_CACHE["nc"] = build()
    return _NC_CACHE["nc"]


def kernel(inputs, mask, W, b, context):
    X = np.asarray(inputs, dtype=np.float32)
    mask = np.asarray(mask)
    W = np.asarray(W, dtype=np.float32)
    b = np.asarray(b, dtype=np.float32)
    context = np.asarray(context, dtype=np.float32)

    nc = _get_nc()

    xt_full = np.ascontiguousarray(X.transpose(0, 2, 1)).reshape(B, KC, P, S).astype(BF)
    xn_full = np.ascontiguousarray(
        X.reshape(B, NXT, 4 * GPH, P, H).transpose(0, 1, 3, 2, 4)
    ).astype(BF)
    wt = np.ascontiguousarray(W.T).reshape(KC, P, H).astype(BF)
    bias_dev = np.ascontiguousarray(b.reshape(MC, P).T)
    ctx_dev = np.ascontiguousarray(context.reshape(MC, P).T).astype(BF)
    mask_f = mask.astype(BF)

    in_maps = []
    for c in range(N_CORES):
        in_maps.append(
            {
                "xt": xt_full[c * BPC : (c + 1) * BPC],
                "xn": xn_full[c * BPC : (c + 1) * BPC],
                "wt": wt,
                "bias": bias_dev,
                "ctx": ctx_dev,
                "mask": mask_f[c * BPC : (c + 1) * BPC],
            }
        )

    res = run_bass_kernel_spmd(nc, in_maps, core_ids=list(range(N_CORES)), trace=TRACE)
    LAST["exec_time_ns"] = res.exec_time_ns
    LAST["result"] = res

    out = np.empty((B, H), np.float32)
    for c in range(N_CORES):
        num = res.results[c]["num"].sum(axis=1)
        den = res.results[c]["den"].sum(axis=1)
        out[c * BPC : (c + 1) * BPC] = num / den[:, None]
    return out



# revision 2
# speedup vs baseline: 1.2028x; 1.2028x over previous
"""Trainium2 Bass kernel for nn_AttnNet: attention-pooling over sequence.

Reference computation (per batch b):
    act    = tanh(X @ W.T + b)          # [S, H]
    scores = act @ context              # [S]
    w      = exp(scores * mask)         # masked_fill(-1e-32) == *mask (exp(0)=1)
    out    = (X.T @ w) / sum(w)         # [H]

Sharding: pure data-parallel, 4 batches per core across 8 cores.

Device layout (per core), X data in bf16:
    xt   [BPC, KC, 128, S]      bf16  xt[b,k,p,s] = X[b, s, 128k+p]  (X^T, h on partitions)
    xn   [BPC, NXT, 128, 16, H] bf16  (X natural, s on partitions, per 2048-seq half)
    wt   [KC, 128, H]           bf16  wt[k,p,o]   = W[o, 128k+p]     (W^T)
    bias [128, MC]              f32   bias[p,m]   = b[128m+p]
    ctx  [128, MC]              bf16  ctx[p,m]    = context[128m+p]
    maskc[BPC, 128, NCH]        f32   maskc[b,p,c] = mask[b, 128c+p] (column layout)
outputs:
    num  [BPC, 4, H]   f32  4 col-group partial pooled rows (host: sum axis=1, divide)
    den  [128, BPC]    f32  per-partition partial softmax denominators (host: sum axis 0)

Pipeline per batch (4 groups g of 1024 seq):
    PE : for m: psum[128,1024] = sum_k wt[k,m]^T @ xt[k]   (8 MMs per (g,m))
    ACT: act[:,m,:] = tanh(psum + bias[m])                 FD=1024, per-m bias
    PE : scores col-chunk MMs: lhsT=act block [128h,128s] (stationary), rhs=ctx[m]
         out = scores_ps[128s, chunk] accumulated over m   (N=1 MMs, column layout)
    DVE: msk = scores[:, :32] * maskc  (one [128,32] op per batch)
    ACT: w = exp(msk) -> bf16, accum_out -> den column     (one [128,32] op per batch)
    PE : pooling col-tiled: pool_ps[32j] += w[:,c]^T @ xn[chunk]   (4 chunks/wave)
    DVE: copy pool rows psum->sbuf, DMA out
Score/exp/pool work for a group/batch is interleaved one group late into the
next group's GEMM stream so the PE never waits on ACT/DVE latency.
"""

from collections import deque

import numpy as np
import ml_dtypes

import concourse.bass as bass
import concourse.tile as tile
from concourse import bacc, mybir
from concourse.bass_utils import run_bass_kernel_spmd

N_CORES = 8
B, S, H = 32, 4096, 512
BPC = B // N_CORES
P = 128
KC = H // P          # 4 contraction blocks
MC = H // P          # 4 output blocks
NCH = S // P         # 32 s-chunks per batch
NXT = 2              # halves (2048 seq each) for xt/xn tiling
GRP = 1024           # seq extent of one GEMM group
NG = S // GRP        # 4 groups per batch
CPG = GRP // P       # 8 chunks per group

F32 = mybir.dt.float32
BF16 = mybir.dt.bfloat16
BF = ml_dtypes.bfloat16

TRACE = False
LAST = {}


def build():
    nc = bacc.Bacc("TRN2", target_bir_lowering=False, num_devices=N_CORES)
    xt_d = nc.declare_dram_parameter("xt", [BPC, KC, P, S], BF16, isOutput=False)
    xn_d = nc.declare_dram_parameter("xn", [BPC, NXT, P, 16, H], BF16, isOutput=False)
    wt_d = nc.declare_dram_parameter("wt", [KC, P, H], BF16, isOutput=False)
    bias_d = nc.declare_dram_parameter("bias", [P, MC], F32, isOutput=False)
    ctx_d = nc.declare_dram_parameter("ctx", [P, MC], BF16, isOutput=False)
    maskc_d = nc.declare_dram_parameter("maskc", [BPC, P, NCH], F32, isOutput=False)
    num_d = nc.declare_dram_parameter("num", [BPC, 4, H], F32, isOutput=True)
    den_d = nc.declare_dram_parameter("den", [P, BPC], F32, isOutput=True)

    Tanh = mybir.ActivationFunctionType.Tanh
    Exp = mybir.ActivationFunctionType.Exp

    with tile.TileContext(nc) as tc:
        with (
            tc.tile_pool(name="singles", bufs=1) as singles,
            tc.tile_pool(name="xtp", bufs=3) as xtp,
            tc.tile_pool(name="xnp", bufs=4) as xnp,
            tc.tile_pool(name="actpool", bufs=3) as actpool,
            tc.tile_pool(name="maskpool", bufs=2) as maskpool,
            tc.tile_pool(name="mskres", bufs=2) as mskres,
            tc.tile_pool(name="wpool", bufs=2) as wpool,
            tc.tile_pool(name="nrp", bufs=2) as nrp,
            tc.tile_pool(name="actps", bufs=2, space="PSUM") as actps,
            tc.tile_pool(name="scps", bufs=2, space="PSUM") as scps,
            tc.tile_pool(name="poolps", bufs=2, space="PSUM") as poolps,
        ):
            wt_sb = singles.tile([P, KC, H], BF16)
            for k in range(KC):
                nc.sync.dma_start(out=wt_sb[:, k, :], in_=wt_d.ap()[k])
            ctx_sb = singles.tile([P, MC], BF16)
            nc.sync.dma_start(out=ctx_sb[:, :], in_=ctx_d.ap())
            bias_sb = singles.tile([P, MC], F32)
            nc.sync.dma_start(out=bias_sb[:, :], in_=bias_d.ap())
            den_sb = singles.tile([P, BPC], F32)

            # deferred work items, interleaved into the PE stream one group late
            items = deque()

            def pop_items(n):
                for _ in range(n):
                    if not items:
                        return
                    items.popleft()()

            def make_chunks(act_sb, sc_ps, g):
                def emit(act=act_sb, sc=sc_ps, gg=g):
                    for cc in range(CPG):
                        c = gg * CPG + cc
                        for m in range(MC):
                            nc.tensor.matmul(
                                sc[:, c : c + 1],
                                lhsT=act[:, m, cc * P : (cc + 1) * P],
                                rhs=ctx_sb[:, m : m + 1],
                                start=(m == 0),
                                stop=(m == MC - 1),
                            )
                return emit

            def make_finish(sc_ps, mask_sb, w_sb, b):
                def emit(sc=sc_ps, msk_in=mask_sb, w=w_sb, bb=b):
                    msk = mskres.tile([P, NCH], F32, tag="msk")
                    nc.vector.tensor_mul(msk[:, :], sc[:, 0:NCH], msk_in[:, :])
                    nc.scalar.activation(
                        out=w[:, :],
                        in_=msk[:, :],
                        func=Exp,
                        accum_out=den_sb[:, bb : bb + 1],
                    )
                return emit

            def make_wave(w_sb, pool_ps, xn_tiles, wv):
                def emit(w=w_sb, pps=pool_ps, xns=xn_tiles, wave=wv):
                    for j in range(4):
                        c = wave * 4 + j
                        nc.tensor.matmul(
                            pps[32 * j : 32 * j + 1, :],
                            lhsT=w[:, c : c + 1],
                            rhs=xns[c // 16][:, c % 16, :],
                            start=(wave == 0),
                            stop=(wave == 7),
                            tile_position=(0, 32 * j),
                            skip_group_check=True,
                        )
                return emit

            def make_numcopy(pool_ps, b):
                def emit(pps=pool_ps, bb=b):
                    nr = nrp.tile([P, H], F32, tag="nr")
                    for j in range(4):
                        nc.vector.tensor_copy(
                            nr[32 * j : 32 * j + 1, :], pps[32 * j : 32 * j + 1, :]
                        )
                        nc.sync.dma_start(
                            out=num_d.ap()[bb, j : j + 1, :],
                            in_=nr[32 * j : 32 * j + 1, :],
                        )
                return emit

            for b in range(BPC):
                mask_sb = maskpool.tile([P, NCH], F32, tag="mask")
                nc.sync.dma_start(out=mask_sb[:, :], in_=maskc_d.ap()[b])
                sc_ps = scps.tile([P, 512], F32, tag="sc")
                pool_ps = poolps.tile([P, 512], F32, tag="pool")
                w_sb = wpool.tile([P, NCH], BF16, tag="w")
                xn_tiles = []

                for half in range(NXT):
                    xt_sb = xtp.tile([P, KC, S // NXT], BF16, tag="xt")
                    if b == 0 and half == 0:
                        for blk in range(4):
                            for k in range(KC):
                                nc.sync.dma_start(
                                    out=xt_sb[:, k, blk * 512 : (blk + 1) * 512],
                                    in_=xt_d.ap()[b, k, :, blk * 512 : (blk + 1) * 512],
                                )
                    else:
                        for k in range(KC):
                            nc.sync.dma_start(
                                out=xt_sb[:, k, :],
                                in_=xt_d.ap()[b, k, :, half * 2048 : (half + 1) * 2048],
                            )
                    xn_sb = xnp.tile([P, 16, H], BF16, tag="xn")
                    nc.sync.dma_start(out=xn_sb[:, :, :], in_=xn_d.ap()[b, half])
                    xn_tiles.append(xn_sb)

                    for gl in range(NG // NXT):
                        g = half * (NG // NXT) + gl
                        act_sb = actpool.tile([P, MC, GRP], BF16, tag="act")
                        for m in range(MC):
                            ps = actps.tile([P, GRP], F32, tag="ps")
                            for hb in range(2):
                                for k in range(KC):
                                    nc.tensor.matmul(
                                        ps[:, hb * 512 : (hb + 1) * 512],
                                        lhsT=wt_sb[:, k, m * P : (m + 1) * P],
                                        rhs=xt_sb[
                                            :,
                                            k,
                                            gl * GRP + hb * 512 : gl * GRP + (hb + 1) * 512,
                                        ],
                                        start=(k == 0),
                                        stop=(k == KC - 1),
                                    )
                            nc.scalar.activation(
                                out=act_sb[:, m, :],
                                in_=ps[:, :],
                                func=Tanh,
                                bias=bias_sb[:, m : m + 1],
                            )
                            pop_items(3)
                        items.append(make_chunks(act_sb, sc_ps, g))

                items.append(make_finish(sc_ps, mask_sb, w_sb, b))
                for wv in range(8):
                    items.append(make_wave(w_sb, pool_ps, xn_tiles, wv))
                items.append(make_numcopy(pool_ps, b))

            while items:
                pop_items(1)
            nc.sync.dma_start(out=den_d.ap()[:, :], in_=den_sb[:, :])

    nc.compile()
    return nc


_NC_CACHE = {}


def _get_nc():
    if "nc" not in _NC_CACHE:
        _NC_CACHE["nc"] = build()
    return _NC_CACHE["nc"]


def kernel(inputs, mask, W, b, context):
    X = np.asarray(inputs, dtype=np.float32)
    mask = np.asarray(mask)
    W = np.asarray(W, dtype=np.float32)
    b = np.asarray(b, dtype=np.float32)
    context = np.asarray(context, dtype=np.float32)

    nc = _get_nc()

    xt_full = np.ascontiguousarray(X.transpose(0, 2, 1)).reshape(B, KC, P, S).astype(BF)
    xn_full = np.ascontiguousarray(
        X.reshape(B, NXT, 16, P, H).transpose(0, 1, 3, 2, 4)
    ).astype(BF)
    wt = np.ascontiguousarray(W.T).reshape(KC, P, H).astype(BF)
    bias_dev = np.ascontiguousarray(b.reshape(MC, P).T)
    ctx_dev = np.ascontiguousarray(context.reshape(MC, P).T).astype(BF)
    maskc = np.ascontiguousarray(
        mask.reshape(B, NCH, P).transpose(0, 2, 1)
    ).astype(np.float32)

    in_maps = []
    for c in range(N_CORES):
        in_maps.append(
            {
                "xt": xt_full[c * BPC : (c + 1) * BPC],
                "xn": xn_full[c * BPC : (c + 1) * BPC],
                "wt": wt,
                "bias": bias_dev,
                "ctx": ctx_dev,
                "maskc": maskc[c * BPC : (c + 1) * BPC],
            }
        )

    res = run_bass_kernel_spmd(nc, in_maps, core_ids=list(range(N_CORES)), trace=TRACE)
    LAST["exec_time_ns"] = res.exec_time_ns
    LAST["result"] = res

    out = np.empty((B, H), np.float32)
    for c in range(N_CORES):
        num = res.results[c]["num"].sum(axis=1)
        den = res.results[c]["den"].sum(axis=0)
        out[c * BPC : (c + 1) * BPC] = num / den[:, None]
    return out
